# revision 3
# baseline (speedup 1.0000x reference)
"""Trainium2 Bass kernel for nn_DynamicResolutionAttention.

B=2, T=2048, C=1024, H=16 heads, head_dim=64.
  q/k/v = x @ W{q,k,v}.T + b     (per-head views)
  attn  = softmax(q k^T / sqrt(hd) * (0.5 + 0.5*resolve))
  y     = attn @ v ; out = y @ Wp.T + bp

Sharding (8 cores): core c = (batch b=c//4, head-group hg=c%4, 4 heads each).
Per core: QKV projections for its 4 heads (transpose-free d-major layouts,
host-pretransposed x^T / W^T), k-major scores S^T = K_h Q_h^T so softmax
denominators come from an appended ones-column on V and no on-chip transpose
is ever needed, exp on the Scalar engine with the runtime temperature,
AllGather of y^T within each batch's 4 cores, then each core computes the
output projection restricted to its own 256 output channels (column split ->
no all-reduce). Host reassembles [B,T,C] from the 8 [T,256] column slices.

All matmuls run as float32r (full-rate fp32 PE mode, ~1e-4 relative rounding).
"""

import sys

for _p in ("/opt/trn_rl_repo",):
    if _p not in sys.path:
        sys.path.insert(0, _p)

import numpy as np

B, T, C, H = 2, 2048, 1024, 16
HD = C // H            # 64
NCORES = 8
HL = 4                 # heads per core
NP = HL // 2           # head pairs per core
CL = HL * HD           # 256 local channels
CIN = C // 128         # 8 contraction tiles
KT_TILES = T // 128    # 16
QC = T // 512          # 4 query chunks

_prog_cache = {}


def _build_program():
    import concourse.mybir as mybir
    import concourse.tile as tile
    from concourse import bacc

    f32 = mybir.dt.float32
    f32r = mybir.dt.float32r

    nc = bacc.Bacc("TRN2", target_bir_lowering=False, debug=False,
                   num_devices=NCORES)

    xT = nc.dram_tensor("xT", [C, T], f32, kind="ExternalInput")
    wqT = nc.dram_tensor("wqT", [C, CL], f32, kind="ExternalInput")
    wkT = nc.dram_tensor("wkT", [C, CL], f32, kind="ExternalInput")
    wvT = nc.dram_tensor("wvT", [C, CL], f32, kind="ExternalInput")
    wpT = nc.dram_tensor("wpT", [C, CL], f32, kind="ExternalInput")
    bq = nc.dram_tensor("bq", [1, CL], f32, kind="ExternalInput")
    bk = nc.dram_tensor("bk", [1, CL], f32, kind="ExternalInput")
    bv = nc.dram_tensor("bv", [1, CL], f32, kind="ExternalInput")
    bp = nc.dram_tensor("bp", [1, CL], f32, kind="ExternalInput")
    rlv = nc.dram_tensor("rlv", [1, 1], f32, kind="ExternalInput")
    ones_d = nc.dram_tensor("ones_d", [1, 512], f32, kind="ExternalInput")
    z = nc.dram_tensor("z", [T, CL], f32, kind="ExternalOutput")

    with tile.TileContext(nc) as tc:
        with tc.tile_pool(name="const", bufs=1) as const, \
             tc.tile_pool(name="big", bufs=1) as big, \
             tc.tile_pool(name="xp", bufs=2) as xp, \
             tc.tile_pool(name="work", bufs=3) as work, \
             tc.tile_pool(name="ps", bufs=2, space="PSUM") as ps, \
             tc.tile_pool(name="dram", bufs=1, space="DRAM") as dram:

            # runtime softmax scale: (0.5 + 0.5*resolve) / sqrt(hd)
            st = const.tile([128, 1], f32)
            nc.sync.dma_start(st[:], rlv[:].to_broadcast((128, 1)))
            nc.vector.tensor_scalar(st[:], st[:], 0.0625, 0.0625,
                                    mybir.AluOpType.mult, mybir.AluOpType.add)

            ones512 = const.tile([1, 512], f32r)
            nc.sync.dma_start(ones512[:], ones_d[:].bitcast(f32r))
            ones128 = const.tile([1, 128], f32r)
            nc.sync.dma_start(ones128[:], ones_d[:, 0:128].bitcast(f32r))
            ones64 = const.tile([1, 64], f32)
            nc.sync.dma_start(ones64[:], ones_d[:, 0:64])

            bq_sb = const.tile([1, CL], f32r)
            bk_sb = const.tile([1, CL], f32r)
            bv_sb = const.tile([1, CL], f32r)
            bp_sb = const.tile([1, CL], f32r)
            nc.sync.dma_start(bq_sb[:], bq[:].bitcast(f32r))
            nc.sync.dma_start(bk_sb[:], bk[:].bitcast(f32r))
            nc.sync.dma_start(bv_sb[:], bv[:].bitcast(f32r))
            nc.sync.dma_start(bp_sb[:], bp[:].bitcast(f32r))

            wq_sb = big.tile([128, CIN, CL], f32r)
            wk_sb = big.tile([128, CIN, CL], f32r)
            wv_sb = big.tile([128, CIN, CL], f32r)
            wp_sb = big.tile([128, CIN, CL], f32r)
            for w_sb, w_dram in ((wq_sb, wqT), (wk_sb, wkT), (wv_sb, wvT)):
                w3 = w_dram[:].rearrange("(o p) c -> p o c", p=128).bitcast(f32r)
                for ci in range(CIN):
                    nc.sync.dma_start(w_sb[:, ci, :], w3[:, ci, :])

            QT = big.tile([128, NP, T], f32r)
            KT = big.tile([128, NP, T], f32r)
            V = big.tile([128, KT_TILES, HL, HD + 1], f32r)
            nc.sync.dma_start(
                V[:, :, :, HD].rearrange("p a b -> p (a b)"),
                ones_d[0:1, 0:1].bitcast(f32r)
                      .to_broadcast((128, KT_TILES * HL)))

            # ---- phase 1: QKV projections, processed in two k-halves ----
            xT3 = xT[:].rearrange("(o p) t -> p o t", p=128).bitcast(f32r)
            for kh in range(2):
                t0 = kh * (T // 2)
                xs = xp.tile([128, CIN, T // 2], f32r, tag="xT")
                for ci in range(CIN):
                    nc.sync.dma_start(xs[:, ci, :], xT3[:, ci, t0:t0 + T // 2])

                for w_sb, b_sb, OUT in ((wq_sb, bq_sb, QT), (wk_sb, bk_sb, KT)):
                    for pair in range(NP):
                        pc = slice(pair * 128, (pair + 1) * 128)
                        for ch in range(2):
                            pm = ps.tile([128, 512], f32, tag="mm")
                            nc.tensor.matmul(pm[:], b_sb[:, pc], ones512[:],
                                             start=True, stop=False)
                            for ci in range(CIN):
                                nc.tensor.matmul(
                                    pm[:], w_sb[:, ci, pc],
                                    xs[:, ci, ch * 512:(ch + 1) * 512],
                                    start=False, stop=(ci == CIN - 1))
                            nc.vector.tensor_copy(
                                OUT[:, pair, t0 + ch * 512:t0 + (ch + 1) * 512],
                                pm[:])

                for tt in range(8):
                    pv = ps.tile([128, CL], f32, tag="mm")
                    nc.tensor.matmul(pv[:], ones128[:], bv_sb[:],
                                     start=True, stop=False)
                    for ci in range(CIN):
                        nc.tensor.matmul(
                            pv[:], xs[:, ci, tt * 128:(tt + 1) * 128],
                            wv_sb[:, ci, :],
                            start=False, stop=(ci == CIN - 1))
                    nc.vector.tensor_copy(
                        V[:, kh * 8 + tt, :, 0:HD],
                        pv[:].rearrange("p (h d) -> p h d", h=HL))

            # wp loads can overlap attention
            wp3 = wpT[:].rearrange("(o p) c -> p o c", p=128).bitcast(f32r)
            for ci in range(CIN):
                nc.sync.dma_start(wp_sb[:, ci, :], wp3[:, ci, :])

            ag_in = dram.tile([CL, T], f32)
            ag_out = dram.tile([4, CL, T], f32)

            # ---- phase 2: attention (k-major S^T, ones-column denominators) --
            for h in range(HL):
                pair, off = h // 2, (h % 2) * HD
                for qc in range(QC):
                    qs = slice(qc * 512, (qc + 1) * 512)
                    py = ps.tile([HD + 1, 512], f32, tag="y")
                    for kt in range(KT_TILES):
                        pss = ps.tile([128, 512], f32, tag="s")
                        nc.tensor.matmul(
                            pss[:],
                            KT[off:off + HD, pair, kt * 128:(kt + 1) * 128],
                            QT[off:off + HD, pair, qs],
                            start=True, stop=True)
                        pt = work.tile([128, 512], f32r, tag="pt")
                        nc.scalar.activation(pt[:], pss[:],
                                             mybir.ActivationFunctionType.Exp,
                                             scale=st[:])
                        nc.tensor.matmul(py[:], V[:, kt, h, :], pt[:],
                                         start=(kt == 0),
                                         stop=(kt == KT_TILES - 1))
                    rec = work.tile([1, 512], f32, tag="rec")
                    nc.vector.reciprocal(rec[:], py[HD:HD + 1, :])
                    pb = ps.tile([HD, 512], f32, tag="b")
                    nc.tensor.matmul(pb[:], ones64[:], rec[:],
                                     start=True, stop=True)
                    pbs = work.tile([HD, 512], f32, tag="pbs")
                    nc.scalar.copy(pbs[:], pb[:])
                    yt = work.tile([HD, 512], f32, tag="yt")
                    nc.vector.tensor_mul(yt[:], py[0:HD, :], pbs[:])
                    nc.sync.dma_start(ag_in[h * HD:(h + 1) * HD, qs], yt[:])

            # ---- phase 3: gather all heads' y^T within the batch group ----
            nc.gpsimd.collective_compute(
                "AllGather", mybir.AluOpType.bypass,
                replica_groups=[[0, 1, 2, 3], [4, 5, 6, 7]],
                ins=[ag_in.opt()], outs=[ag_out.opt()])

            # ---- phase 4: output projection, this core's 256 columns ----
            ag_flat = ag_out[:].rearrange("g c t -> (g c) t") \
                               .rearrange("(o p) t -> p o t", p=128) \
                               .bitcast(f32r)
            for tt in range(KT_TILES):
                ts = slice(tt * 128, (tt + 1) * 128)
                ys = work.tile([128, CIN, 128], f32r, tag="ys")
                nc.sync.dma_start(ys[:], ag_flat[:, :, ts])
                pz = ps.tile([128, CL], f32, tag="mm")
                nc.tensor.matmul(pz[:], ones128[:], bp_sb[:],
                                 start=True, stop=False)
                for ci in range(CIN):
                    nc.tensor.matmul(pz[:], ys[:, ci, :], wp_sb[:, ci, :],
                                     start=False, stop=(ci == CIN - 1))
                zs = work.tile([128, CL], f32, tag="zs")
                nc.vector.tensor_copy(zs[:], pz[:])
                nc.sync.dma_start(z[ts, :], zs[:])

    nc.compile()
    return nc


def _get_program():
    if "nc" not in _prog_cache:
        _prog_cache["nc"] = _build_program()
    return _prog_cache["nc"]


def kernel(x, Wq, bq, Wk, bk, Wv, bv, Wp, bp, resolve_level):
    from concourse.bass_utils import run_bass_kernel_spmd

    nc = _get_program()

    x = np.asarray(x, np.float32)
    rl = np.asarray(resolve_level, np.float32).reshape(1, 1)

    xT_b = [np.ascontiguousarray(x[b].T) for b in range(B)]
    in_maps = []
    for c in range(NCORES):
        b, hg = c // 4, c % 4
        cs = slice(hg * CL, (hg + 1) * CL)
        in_maps.append({
            "xT": xT_b[b],
            "wqT": np.ascontiguousarray(np.asarray(Wq, np.float32)[cs, :].T),
            "wkT": np.ascontiguousarray(np.asarray(Wk, np.float32)[cs, :].T),
            "wvT": np.ascontiguousarray(np.asarray(Wv, np.float32)[cs, :].T),
            "wpT": np.ascontiguousarray(np.asarray(Wp, np.float32)[cs, :].T),
            "bq": np.asarray(bq, np.float32)[cs].reshape(1, CL).copy(),
            "bk": np.asarray(bk, np.float32)[cs].reshape(1, CL).copy(),
            "bv": np.asarray(bv, np.float32)[cs].reshape(1, CL).copy(),
            "bp": np.asarray(bp, np.float32)[cs].reshape(1, CL).copy(),
            "rlv": rl,
            "ones_d": np.ones((1, 512), np.float32),
        })

    res = run_bass_kernel_spmd(nc, in_maps, core_ids=list(range(NCORES)))

    out = np.empty((B, T, C), np.float32)
    for c in range(NCORES):
        b, hg = c // 4, c % 4
        out[b, :, hg * CL:(hg + 1) * CL] = res.results[c]["z"]
    return out


# revision 4
# speedup vs baseline: 1.3296x; 1.3296x over previous
"""Trainium2 Bass kernel for nn_DynamicResolutionAttention.

B=2, T=2048, C=1024, H=16 heads, head_dim=64.
  q/k/v = x @ W{q,k,v}.T + b     (per-head views)
  attn  = softmax(q k^T / sqrt(hd) * (0.5 + 0.5*resolve))
  y     = attn @ v ; out = y @ Wp.T + bp

Sharding (8 cores): core c = (batch b=c//4, head-group hg=c%4, 4 heads each).
Per core: QKV projections for its 4 heads (transpose-free d-major layouts,
host-pretransposed x^T / W^T), k-major scores S^T = K_h Q_h^T so softmax
denominators come from an appended ones-column on V and no on-chip transpose
is ever needed, exp on the Scalar engine with the runtime temperature,
AllGather of y^T within each batch's 4 cores, then each core computes the
output projection restricted to its own 256 output channels (column split ->
no all-reduce). Host reassembles [B,T,C] from the 8 [T,256] column slices.

Matmul operands are bf16 (fp32 PSUM accumulation); softmax statistics stay fp32.
"""

import sys

for _p in ("/opt/trn_rl_repo",):
    if _p not in sys.path:
        sys.path.insert(0, _p)

import numpy as np

B, T, C, H = 2, 2048, 1024, 16
HD = C // H            # 64
NCORES = 8
HL = 4                 # heads per core
NP = HL // 2           # head pairs per core
CL = HL * HD           # 256 local channels
CIN = C // 128         # 8 contraction tiles
KT_TILES = T // 128    # 16
QC = T // 512          # 4 query chunks

_prog_cache = {}


def _build_program():
    import concourse.mybir as mybir
    import concourse.tile as tile
    from concourse import bacc

    f32 = mybir.dt.float32
    bf16 = mybir.dt.bfloat16

    nc = bacc.Bacc("TRN2", target_bir_lowering=False, debug=False,
                   num_devices=NCORES)

    xT = nc.dram_tensor("xT", [C, T], bf16, kind="ExternalInput")
    wqT = nc.dram_tensor("wqT", [C, CL], bf16, kind="ExternalInput")
    wkT = nc.dram_tensor("wkT", [C, CL], bf16, kind="ExternalInput")
    wvT = nc.dram_tensor("wvT", [C, CL], bf16, kind="ExternalInput")
    wpT = nc.dram_tensor("wpT", [C, CL], bf16, kind="ExternalInput")
    bq = nc.dram_tensor("bq", [1, CL], bf16, kind="ExternalInput")
    bk = nc.dram_tensor("bk", [1, CL], bf16, kind="ExternalInput")
    bv = nc.dram_tensor("bv", [1, CL], bf16, kind="ExternalInput")
    bp = nc.dram_tensor("bp", [1, CL], bf16, kind="ExternalInput")
    rlv = nc.dram_tensor("rlv", [1, 1], f32, kind="ExternalInput")
    ones_d = nc.dram_tensor("ones_d", [1, 512], bf16, kind="ExternalInput")
    z = nc.dram_tensor("z", [T, CL], f32, kind="ExternalOutput")

    with tile.TileContext(nc) as tc:
        with tc.tile_pool(name="const", bufs=1) as const, \
             tc.tile_pool(name="big", bufs=1) as big, \
             tc.tile_pool(name="xp", bufs=2) as xp, \
             tc.tile_pool(name="work", bufs=3) as work, \
             tc.tile_pool(name="ps", bufs=2, space="PSUM") as ps, \
             tc.tile_pool(name="dram", bufs=1, space="DRAM") as dram:

            # runtime softmax scale: (0.5 + 0.5*resolve) / sqrt(hd)
            st = const.tile([128, 1], f32)
            nc.sync.dma_start(st[:], rlv[:].to_broadcast((128, 1)))
            nc.vector.tensor_scalar(st[:], st[:], 0.0625, 0.0625,
                                    mybir.AluOpType.mult, mybir.AluOpType.add)

            ones512 = const.tile([1, 512], bf16)
            nc.sync.dma_start(ones512[:], ones_d[:])
            ones128 = const.tile([1, 128], bf16)
            nc.sync.dma_start(ones128[:], ones_d[:, 0:128])
            ones64 = const.tile([1, 64], f32)
            nc.vector.memset(ones64[:], 1.0)

            bq_sb = const.tile([1, CL], bf16)
            bk_sb = const.tile([1, CL], bf16)
            bv_sb = const.tile([1, CL], bf16)
            bp_sb = const.tile([1, CL], bf16)
            nc.sync.dma_start(bq_sb[:], bq[:])
            nc.sync.dma_start(bk_sb[:], bk[:])
            nc.sync.dma_start(bv_sb[:], bv[:])
            nc.sync.dma_start(bp_sb[:], bp[:])

            wq_sb = big.tile([128, CIN, CL], bf16)
            wk_sb = big.tile([128, CIN, CL], bf16)
            wv_sb = big.tile([128, CIN, CL], bf16)
            wp_sb = big.tile([128, CIN, CL], bf16)
            for w_sb, w_dram in ((wq_sb, wqT), (wk_sb, wkT), (wv_sb, wvT)):
                w3 = w_dram[:].rearrange("(o p) c -> p o c", p=128)
                for ci in range(CIN):
                    nc.sync.dma_start(w_sb[:, ci, :], w3[:, ci, :])

            QT = big.tile([128, NP, T], bf16)
            KT = big.tile([128, NP, T], bf16)
            V = big.tile([128, KT_TILES, HL, HD + 1], bf16)
            nc.sync.dma_start(
                V[:, :, :, HD].rearrange("p a b -> p (a b)"),
                ones_d[0:1, 0:1].to_broadcast((128, KT_TILES * HL)))

            # ---- phase 1: QKV projections, processed in two k-halves ----
            xT3 = xT[:].rearrange("(o p) t -> p o t", p=128)
            for kh in range(2):
                t0 = kh * (T // 2)
                xs = xp.tile([128, CIN, T // 2], bf16, tag="xT")
                for ci in range(CIN):
                    nc.sync.dma_start(xs[:, ci, :], xT3[:, ci, t0:t0 + T // 2])

                for w_sb, b_sb, OUT in ((wq_sb, bq_sb, QT), (wk_sb, bk_sb, KT)):
                    for pair in range(NP):
                        pc = slice(pair * 128, (pair + 1) * 128)
                        for ch in range(2):
                            pm = ps.tile([128, 512], f32, tag="mm")
                            nc.tensor.matmul(pm[:], b_sb[:, pc], ones512[:],
                                             start=True, stop=False)
                            for ci in range(CIN):
                                nc.tensor.matmul(
                                    pm[:], w_sb[:, ci, pc],
                                    xs[:, ci, ch * 512:(ch + 1) * 512],
                                    start=False, stop=(ci == CIN - 1))
                            nc.vector.tensor_copy(
                                OUT[:, pair, t0 + ch * 512:t0 + (ch + 1) * 512],
                                pm[:])

                for tt in range(8):
                    pv = ps.tile([128, CL], f32, tag="mm")
                    nc.tensor.matmul(pv[:], ones128[:], bv_sb[:],
                                     start=True, stop=False)
                    for ci in range(CIN):
                        nc.tensor.matmul(
                            pv[:], xs[:, ci, tt * 128:(tt + 1) * 128],
                            wv_sb[:, ci, :],
                            start=False, stop=(ci == CIN - 1))
                    nc.vector.tensor_copy(
                        V[:, kh * 8 + tt, :, 0:HD],
                        pv[:].rearrange("p (h d) -> p h d", h=HL))

            # wp loads can overlap attention
            wp3 = wpT[:].rearrange("(o p) c -> p o c", p=128)
            for ci in range(CIN):
                nc.sync.dma_start(wp_sb[:, ci, :], wp3[:, ci, :])

            ag_in = dram.tile([CL, T], bf16)
            ag_out = dram.tile([4, CL, T], bf16)

            # ---- phase 2: attention (k-major S^T, ones-column denominators) --
            for h in range(HL):
                pair, off = h // 2, (h % 2) * HD
                for qc in range(QC):
                    qs = slice(qc * 512, (qc + 1) * 512)
                    py = ps.tile([HD + 1, 512], f32, tag="y")
                    for kt in range(KT_TILES):
                        pss = ps.tile([128, 512], f32, tag="s")
                        nc.tensor.matmul(
                            pss[:],
                            KT[off:off + HD, pair, kt * 128:(kt + 1) * 128],
                            QT[off:off + HD, pair, qs],
                            start=True, stop=True)
                        pt = work.tile([128, 512], bf16, tag="pt")
                        nc.scalar.activation(pt[:], pss[:],
                                             mybir.ActivationFunctionType.Exp,
                                             scale=st[:])
                        nc.tensor.matmul(py[:], V[:, kt, h, :], pt[:],
                                         start=(kt == 0),
                                         stop=(kt == KT_TILES - 1))
                    rec = work.tile([1, 512], f32, tag="rec")
                    nc.vector.reciprocal(rec[:], py[HD:HD + 1, :])
                    pb = ps.tile([HD, 512], f32, tag="b")
                    nc.tensor.matmul(pb[:], ones64[:], rec[:],
                                     start=True, stop=True)
                    pbs = work.tile([HD, 512], f32, tag="pbs")
                    nc.vector.tensor_copy(pbs[:], pb[:])
                    yt = work.tile([HD, 512], bf16, tag="yt")
                    nc.vector.tensor_mul(yt[:], py[0:HD, :], pbs[:])
                    nc.sync.dma_start(ag_in[h * HD:(h + 1) * HD, qs], yt[:])

            # ---- phase 3: gather all heads' y^T within the batch group ----
            nc.gpsimd.collective_compute(
                "AllGather", mybir.AluOpType.bypass,
                replica_groups=[[0, 1, 2, 3], [4, 5, 6, 7]],
                ins=[ag_in.opt()], outs=[ag_out.opt()])

            # ---- phase 4: output projection, this core's 256 columns ----
            ag_flat = ag_out[:].rearrange("g c t -> (g c) t") \
                               .rearrange("(o p) t -> p o t", p=128)
            for tt in range(KT_TILES):
                ts = slice(tt * 128, (tt + 1) * 128)
                ys = work.tile([128, CIN, 128], bf16, tag="ys")
                nc.sync.dma_start(ys[:], ag_flat[:, :, ts])
                pz = ps.tile([128, CL], f32, tag="mm")
                nc.tensor.matmul(pz[:], ones128[:], bp_sb[:],
                                 start=True, stop=False)
                for ci in range(CIN):
                    nc.tensor.matmul(pz[:], ys[:, ci, :], wp_sb[:, ci, :],
                                     start=False, stop=(ci == CIN - 1))
                zs = work.tile([128, CL], f32, tag="zs")
                nc.vector.tensor_copy(zs[:], pz[:])
                nc.sync.dma_start(z[ts, :], zs[:])

    nc.compile()
    return nc


def _get_program():
    if "nc" not in _prog_cache:
        _prog_cache["nc"] = _build_program()
    return _prog_cache["nc"]


def kernel(x, Wq, bq, Wk, bk, Wv, bv, Wp, bp, resolve_level):
    import ml_dtypes
    from concourse.bass_utils import run_bass_kernel_spmd

    bfl = ml_dtypes.bfloat16
    nc = _get_program()

    x = np.asarray(x, np.float32)
    rl = np.asarray(resolve_level, np.float32).reshape(1, 1)

    xT_b = [np.ascontiguousarray(x[b].T).astype(bfl) for b in range(B)]
    in_maps = []
    for c in range(NCORES):
        b, hg = c // 4, c % 4
        cs = slice(hg * CL, (hg + 1) * CL)
        in_maps.append({
            "xT": xT_b[b],
            "wqT": np.ascontiguousarray(np.asarray(Wq, np.float32)[cs, :].T).astype(bfl),
            "wkT": np.ascontiguousarray(np.asarray(Wk, np.float32)[cs, :].T).astype(bfl),
            "wvT": np.ascontiguousarray(np.asarray(Wv, np.float32)[cs, :].T).astype(bfl),
            "wpT": np.ascontiguousarray(np.asarray(Wp, np.float32)[cs, :].T).astype(bfl),
            "bq": np.asarray(bq, np.float32)[cs].reshape(1, CL).astype(bfl),
            "bk": np.asarray(bk, np.float32)[cs].reshape(1, CL).astype(bfl),
            "bv": np.asarray(bv, np.float32)[cs].reshape(1, CL).astype(bfl),
            "bp": np.asarray(bp, np.float32)[cs].reshape(1, CL).astype(bfl),
            "rlv": rl,
            "ones_d": np.ones((1, 512), bfl),
        })

    res = run_bass_kernel_spmd(nc, in_maps, core_ids=list(range(NCORES)))

    out = np.empty((B, T, C), np.float32)
    for c in range(NCORES):
        b, hg = c // 4, c % 4
        out[b, :, hg * CL:(hg + 1) * CL] = res.results[c]["z"]
    return out
